# revision 33
# baseline (speedup 1.0000x reference)
"""CPDBlock (rank-decomposed conv block) Trainium2 kernel.

Reference computation (per image):
  y1 = (sum_r w_head[r]) @ x            # 1x1 conv, 256->256
  y2 = conv_(3,1)(y1, w_body)           # 256->64, pad (1,0) in H
  y3 = conv_(1,3)(y2, w_tail) + b_tail  # 64->256, pad (0,1) in W

Algebraic fusion: head folds into body since both are linear:
  y2[r,h,w] = sum_kh (w_body[:, :, kh] @ w_sum) @ x[:, h+kh-1, w]
so the kernel only runs two conv stages:
  fused:  Wc[kh] = w_body[kh] @ w_head.sum(0)  (3x [64,256], host-side)
  tail:   w_tail as-is, bias folded into the PSUM->SBUF drains.

Sharding: data-parallel over batch, 16 images / 8 cores = 2 images/core.

Layout per core, per image, H processed in chunks of HC rows:
  x chunk  [128p=cin%128, 2=cin//128, HC+2 rows (halo), 112]  (SBUF, fp16)
  y2d      [128p, HC rows, 114]: partitions 0-63 hold y2 row-padded
           (col0=0, cols1..112=data), partitions 64-127 hold the same
           shifted one col left (cols0..111=data, col112=0).  This gives
           the tail's three shifted W-windows as plain AP offsets with
           K=128 matmuls (third tap uses a half-zero lhsT).
  y3 stage [128p=cout%128, 2=cout//128, HC, 112] -> DMA out.

Perf structure (v2):
  - y2d pad columns zeroed by DVE memset (a DMA here explodes into
    thousands of 4B packets that stall the input stream for ~15us).
  - x loads issued from the gpsimd queue in 8-row pieces; weights and
    y3 stores go through the sync queue, so input prefetch is never
    stuck behind store descriptors and the first matmul starts early.
  - halo rows are re-read from HBM (2 rows/chunk) instead of copied.
  - the two y2->y2d copies run on ACT and DVE in parallel; the y3
    drains (+bias) alternate mo0->ACT (activation w/ per-partition
    bias), mo1->DVE (tensor_tensor broadcast add).
  - tail matmuls for pair k are emitted after fused matmuls for pair
    k+1 (software pipelining), so the tensor engine never waits on the
    PSUM->SBUF copies.
"""
import os

import numpy as np

import concourse.bass as bass
import concourse.mybir as mybir
import concourse.tile as tile
from concourse import bacc
from concourse.bass_utils import run_bass_kernel_spmd

F32 = mybir.dt.float32
F32R = mybir.dt.float32r
F16 = mybir.dt.float16

B, CIN, COUT, RANK, H, W = 16, 256, 256, 64, 112, 112
NCORES = 8
BL = B // NCORES          # images per core
KO = CIN // 128           # input-channel k-tiles
MO = COUT // 128          # output-channel m-tiles
HC = 56                   # rows per chunk
NCH = H // HC             # chunks per image
NR = 4                    # output rows per matmul group (N = NR*112 = 448)
NG = HC // NR             # groups per chunk

LAST_EXEC_NS = None
LAST_IN_MAPS = None


def _build(reps: int = 1, loop_reps: int = 1, xdt=None, odt=None):
    fp16 = os.environ.get("CPD_FP16", "1") == "1"
    if xdt is None:
        xdt = F16 if fp16 else F32R
    if odt is None:
        odt = F16 if fp16 else F32
    nc = bacc.Bacc("TRN2", target_bir_lowering=False, debug=False,
                   num_devices=NCORES)
    x_d = nc.dram_tensor("x", [BL, CIN, H, W], xdt, kind="ExternalInput")
    wf_d = nc.dram_tensor("wf", [128, 3, KO, RANK], xdt, kind="ExternalInput")
    wt_d = nc.dram_tensor("wt", [128, MO, 2, 128], xdt, kind="ExternalInput")
    bias_d = nc.dram_tensor("bias", [128, MO], F32, kind="ExternalInput")
    o_d = nc.dram_tensor("o", [BL, COUT, H, W], odt, kind="ExternalOutput")

    with tile.TileContext(nc) as tc:
        with (
            tc.tile_pool(name="wpool", bufs=1) as wpool,
            tc.tile_pool(name="xpool", bufs=2) as xpool,
            tc.tile_pool(name="ypool", bufs=1) as ypool,
            tc.tile_pool(name="opool", bufs=2) as opool,
            tc.tile_pool(name="psf", bufs=4, space="PSUM") as psf,
            tc.tile_pool(name="pst", bufs=2, space="PSUM") as pst,
        ):
            wf = wpool.tile([128, 3, KO, RANK], xdt)
            wt = wpool.tile([128, MO, 2, 128], xdt)
            bias = wpool.tile([128, MO], F32)
            # DMA routing.  sync and scalar are the two fast HWDGE rings
            # (gpsimd is the slow SWDGE path -- ~2us completion overhead --
            # so it gets no DMAs at all).  wf is needed by the very first
            # matmul -> sync queue, first.  wt/bias ride the scalar queue
            # behind the first x piece (rings are FIFO, so they cannot
            # steal packet slots from it).  Stores go on sync; x loads for
            # chunk c+1 are issued at the top of chunk c's sync stream so
            # the issue executes a full chunk-period early and is never
            # stuck behind a store's drain-dependency.
            # wf rides the sync ring (the scalar ring's head is blocked by
            # the 1.3us ACT table load, so a scalar-half would gate the
            # first matmul); p0's ko=1 half leads the scalar ring instead.
            nc.sync.dma_start(wf[:], wf_d[:])

            # Two persistent y2d buffers, manually alternated per chunk.
            # Their pad columns (left pad of the top half, right pad of the
            # bottom half) are zeroed once by DVE and never written again.
            y2ds = [ypool.tile([128, HC, 114], xdt, tag=f"y2d{i}",
                               name=f"y2d{i}")
                    for i in range(2)]
            for y2d in y2ds:
                pad_a, pad_b = y2d[0:64, :, 0], y2d[64:128, :, 112]
                if xdt == F32R:
                    pad_a, pad_b = pad_a.bitcast(F32), pad_b.bitcast(F32)
                nc.vector.memset(pad_a, 0.0)
                nc.vector.memset(pad_b, 0.0)

            def emit_tail(pend):
                """Tail matmuls + drains (+bias) + y3 store for one pair.

                Stores are batched to 16 rows (one per two pairs) to halve
                the store packet count; the last pair of a chunk stores its
                own 8 rows, and the final pair of the whole kernel splits
                that store by mo across the sync and gpsimd queues so the
                end-of-kernel drain is shorter.
                """
                y2d, y3t, ov, h0, gp, subs, s_from, smode = pend
                for sub in subs:
                    g = gp + sub
                    r0 = g * NR
                    for mo in range(MO):
                        pt = pst.tile([128, NR, W], F32,
                                      tag=f"pt{mo}", name=f"pt{mo}")
                        nc.tensor.matmul(pt[:], wt[:, mo, 0, :],
                                         y2d[:, r0:r0 + NR, 0:112],
                                         start=True, stop=False)
                        nc.tensor.matmul(pt[:], wt[:, mo, 1, :],
                                         y2d[:, r0:r0 + NR, 1:113],
                                         start=False, stop=True)
                        if mo == 0:
                            nc.scalar.add(y3t[:, mo, r0:r0 + NR, :], pt[:],
                                          bias[:, mo:mo + 1])
                        else:
                            nc.vector.tensor_tensor(
                                y3t[:, mo, r0:r0 + NR, :],
                                pt[:],
                                bias[:, mo, None].to_broadcast(
                                    [128, NR, W]),
                                mybir.AluOpType.add,
                            )
                if s_from is None:
                    return
                r1 = (gp + len(subs)) * NR
                # Stores stay on the HWDGE rings.  (SWDGE/gpsimd is NOT an
                # option: its SBUF descriptor-ring fetches contend with the
                # PE's SBUF read ports and inflate matmul time by ~20%.)
                if smode == "split":
                    # Final pair: 4-row, per-mo stores on both rings so the
                    # first half flows while the second half drains.
                    for sub in subs:
                        ra, rb = (gp + sub) * NR, (gp + sub + 1) * NR
                        nc.sync.dma_start(ov[:, 0:1, h0 + ra:h0 + rb, :],
                                          y3t[:, 0:1, ra:rb, :])
                        nc.scalar.dma_start(ov[:, 1:2, h0 + ra:h0 + rb, :],
                                            y3t[:, 1:2, ra:rb, :])
                elif smode == "scalar":
                    nc.scalar.dma_start(ov[:, :, h0 + s_from:h0 + r1, :],
                                        y3t[:, :, s_from:r1, :])
                else:
                    nc.sync.dma_start(ov[:, :, h0 + s_from:h0 + r1, :],
                                      y3t[:, :, s_from:r1, :])

            def issue_x(b, ch, first, second=False):
                """Allocate + load one chunk's x tile.

                xt slot i holds absolute image row ch*HC + i - 1; edge
                chunks leave the out-of-image slot unwritten and the
                matmul term that would read it is skipped.  Interior
                chunks re-read the 2 halo rows from HBM.  Rows are
                contiguous in HBM, so a piece costs one DMA descriptor
                per (channel, ko) regardless of row count -> steady-state
                chunks load in a single piece.  The first chunk is
                latency-critical: small pieces, split by ko across the
                two HWDGE rings, with wt/bias slotted behind piece0.
                """
                h0 = ch * HC
                xv = x_d.ap()[b].rearrange("(ko p) h w -> p ko h w", p=128)
                xt = xpool.tile([128, KO, HC + 2, W], xdt)
                lo = 1 if ch == 0 else 0
                hi = HC + 2 if ch < NCH - 1 else HC + 1
                if first:
                    def ld(ko, s0, s1, eng):
                        eng.dma_start(
                            xt[:, ko:ko + 1, s0:s1, :],
                            xv[:, ko:ko + 1, h0 + s0 - 1:h0 + s1 - 1, :])
                    # Descriptor count is one per (channel, piece), so the
                    # tail of the chunk loads in a single big piece: same
                    # descriptor latency, 2.5x the data.
                    ld(0, lo, 9, nc.sync)
                    ld(1, lo, 9, nc.scalar)
                    nc.scalar.dma_start(wt[:], wt_d[:])
                    nc.scalar.dma_start(bias[:], bias_d[:])
                    ld(0, 9, 17, nc.sync)
                    ld(1, 9, 17, nc.scalar)
                    ld(0, 17, 33, nc.sync)
                    ld(1, 17, 33, nc.scalar)
                    ld(0, 33, hi, nc.sync)
                    ld(1, 33, hi, nc.scalar)
                elif second:
                    # Still latency-tight (the first chunk's compute is
                    # short): finer pieces so later pairs aren't gated on
                    # one big landing.
                    for s0, s1 in ((lo, 10), (10, 20), (20, 34), (34, hi)):
                        nc.sync.dma_start(
                            xt[:, :, s0:s1, :],
                            xv[:, :, h0 + s0 - 1:h0 + s1 - 1, :])
                else:
                    for s0, s1 in ((lo, 20), (20, hi)):
                        nc.sync.dma_start(
                            xt[:, :, s0:s1, :],
                            xv[:, :, h0 + s0 - 1:h0 + s1 - 1, :])
                return xt

            import collections
            import contextlib
            loop_cm = (tc.For_i(0, loop_reps, 1) if loop_reps > 1
                       else contextlib.nullcontext())
            it = 0
            pendq = collections.deque()
            chunk_seq = [(rep, b, ch) for rep in range(reps)
                         for b in range(BL) for ch in range(NCH)]
            xts = {}
            with loop_cm:
              for ci, (rep, b, ch) in enumerate(chunk_seq):
                    if ci == 0:
                        xts[ci] = issue_x(b, ch, True)
                    if ci + 1 < len(chunk_seq):
                        nb, nch = chunk_seq[ci + 1][1:]
                        xts[ci + 1] = issue_x(nb, nch, False, ci == 0)
                    xt = xts.pop(ci)
                    h0 = ch * HC
                    ov = o_d.ap()[b].rearrange("(mo p) h w -> p mo h w",
                                               p=128)

                    y2d = y2ds[it % 2]
                    it += 1
                    y3t = opool.tile([128, MO, HC, W], odt)

                    # Fused-stage groups are processed in pairs: group gp
                    # lands in PSUM partitions 0:64 (PE column-group 0/1),
                    # group gp+1 in partitions 64:128 (column-group 2/3).
                    # The two col-group matmul streams execute concurrently
                    # in the PE array, halving the fused-stage wall time.
                    for gp in range(0, NG, 2):
                        subs = [0, 1] if gp + 1 < NG else [0]
                        pfp = psf.tile([128, NR, W], F32)
                        for ko in range(KO):
                            for kh in (1, 0, 2):
                                for sub in subs:
                                    g = gp + sub
                                    r0 = g * NR
                                    p0 = 64 * sub
                                    out_ap = pfp[p0:p0 + 64, :]
                                    rhs = xt[:, ko, r0 + kh:r0 + kh + NR, :]
                                    if ch == 0 and g == 0 and kh == 0:
                                        # output row 0 has no row above
                                        out_ap = pfp[p0:p0 + 64, 1:NR, :]
                                        rhs = xt[:, ko, 1:NR, :]
                                    elif (ch == NCH - 1 and g == NG - 1
                                          and kh == 2):
                                        # last row has no row below
                                        out_ap = pfp[p0:p0 + 64, 0:NR - 1, :]
                                        rhs = xt[:, ko, r0 + 2:r0 + 1 + NR, :]
                                    nc.tensor.matmul(
                                        out_ap,
                                        wf[:, kh, ko, :],
                                        rhs,
                                        start=(ko == 0 and kh == 1),
                                        stop=(ko == KO - 1 and kh == 2),
                                        tile_position=(0, p0),
                                    )
                        # y2 -> both halves of the padded/shifted layout,
                        # one on ACT and one on DVE so the tail is never
                        # gated on a single engine's serial copies.
                        for sub in subs:
                            g = gp + sub
                            r0 = g * NR
                            pf = pfp[64 * sub:64 * sub + 64, :]
                            nc.scalar.copy(y2d[0:64, r0:r0 + NR, 1:113], pf)
                            nc.vector.tensor_copy(
                                y2d[64:128, r0:r0 + NR, 0:112], pf)
                        # Tails run two pairs behind the fused stage and
                        # are emitted in blocks of two (f,f,t,t): the PE
                        # pays its split-stream <-> full-width transition
                        # penalty (~95ns) once per block instead of once
                        # per pair, and the y2d copies get two pair-periods
                        # of slack.
                        pi = gp // 2
                        last_chunk = ci == len(chunk_seq) - 1
                        if last_chunk:
                            s_from = gp * NR
                            if gp + 2 >= NG:
                                smode = "split"
                            else:
                                smode = "scalar" if pi % 2 else "sync"
                        elif pi % 2 == 1:
                            s_from, smode = (gp - 2) * NR, "sync"
                        elif gp + 2 >= NG:
                            s_from, smode = gp * NR, "sync"
                        else:
                            s_from, smode = None, None
                        pendq.append(
                            (y2d, y3t, ov, h0, gp, subs, s_from, smode))
                        if len(pendq) >= 2:
                            emit_tail(pendq.popleft())
              while pendq:
                emit_tail(pendq.popleft())
    nc.compile()
    return nc


_NC_CACHE = None


def kernel(x, w_head, w_body, w_tail, b_tail):
    global _NC_CACHE, LAST_EXEC_NS
    x = np.ascontiguousarray(np.asarray(x, dtype=np.float32))
    w_head = np.asarray(w_head, dtype=np.float32)
    w_body = np.asarray(w_body, dtype=np.float32)
    w_tail = np.asarray(w_tail, dtype=np.float32)
    b_tail = np.asarray(b_tail, dtype=np.float32)

    # --- host-side weight prep (tiny) ---
    w_sum = w_head.astype(np.float64).sum(axis=0)          # [COUT, CIN]
    wc = np.einsum("rok,oi->kri", w_body[:, :, :, 0].astype(np.float64),
                   w_sum)                                  # [3, RANK, CIN]
    # wf[p, kh, ko, m] = Wc[kh][m, ko*128+p]
    wf = np.transpose(wc.reshape(3, RANK, KO, 128), (3, 0, 2, 1))
    wf = np.ascontiguousarray(wf.astype(np.float32))

    # wt[p, mo, 0, m]: p<64 -> w_tail[mo*128+m, p, 0, 0]; p>=64 -> tap1
    #   [p, mo, 1, m]: p<64 -> 0;                         p>=64 -> tap2
    wt = np.zeros((128, MO, 2, 128), dtype=np.float32)
    wtl = w_tail[:, :, 0, :].reshape(MO, 128, RANK, 3)     # [mo, m, r, kw]
    wt[0:64, :, 0, :] = np.transpose(wtl[:, :, :, 0], (2, 0, 1))
    wt[64:128, :, 0, :] = np.transpose(wtl[:, :, :, 1], (2, 0, 1))
    wt[64:128, :, 1, :] = np.transpose(wtl[:, :, :, 2], (2, 0, 1))

    bias = np.ascontiguousarray(b_tail.reshape(MO, 128).T)  # [128, mo]

    fp16 = os.environ.get("CPD_FP16", "1") == "1"
    if fp16:
        x = np.ascontiguousarray(x.astype(np.float16))
        wf = np.ascontiguousarray(wf.astype(np.float16))
        wt = np.ascontiguousarray(wt.astype(np.float16))

    if _NC_CACHE is None:
        _NC_CACHE = _build()
    nc = _NC_CACHE

    in_maps = [
        {"x": x[c * BL:(c + 1) * BL], "wf": wf, "wt": wt, "bias": bias}
        for c in range(NCORES)
    ]
    global LAST_IN_MAPS
    LAST_IN_MAPS = in_maps
    trace = os.environ.get("KBENCH_TRACE", "0") == "1"
    res = run_bass_kernel_spmd(nc, in_maps, core_ids=list(range(NCORES)),
                               trace=trace)
    LAST_EXEC_NS = res.exec_time_ns
    out = np.concatenate([r["o"] for r in res.results], axis=0)
    if out.dtype != np.float32:
        out = out.astype(np.float32)
    return out
